# revision 17
# baseline (speedup 1.0000x reference)
"""Distributed Trainium2 kernel for a dense transformer block.

Reference computation (per batch):
  x = x + o_proj(attn(rope(qkv(rmsnorm(x))), causal)) ; x = x + w2(silu(wg(rmsnorm(x))) * w1(rmsnorm(x)))

Sharding: DP=2 on batch x 4-way hybrid within each batch group.
Cores 0-3 handle batch 0, cores 4-7 batch 1. Attention is tensor-parallel
(rank r owns heads 4r..4r+3); a chunked ReduceScatter then hands each rank
ownership of 512 tokens (4 fragments of 128), and the MLP runs
sequence-parallel with full weights streamed from HBM — no second
collective and no final AllGather (the host reassembles output slices).

The PE clock ramps with continuous busy time (0.65 -> 1.2 -> 2.4 GHz), so
independent matmul work (next chunk's qkv, previous chunk's o-proj, first
MLP half-pass) is interleaved into the attention chain's stall points via
a filler queue to keep the tensor engine saturated.
"""

import sys

sys.path.insert(0, "/opt/trn_rl_repo")

import numpy as np
import ml_dtypes

import concourse.bass as bass
import concourse.bacc as bacc
import concourse.mybir as mybir
import concourse.tile as tile
from concourse.bass_utils import run_bass_kernel_spmd

BF = ml_dtypes.bfloat16
F32 = mybir.dt.float32
BF16 = mybir.dt.bfloat16

D = 1024
NH = 16
DH = 64
MULT = 4
EPS = 1e-5
ROPE_BASE = 10000.0
B = 2
TP = 4  # ranks per group
HPC = NH // TP  # heads per core = 4
QKF = 2 * HPC * DH  # q+k shard features = 512
VF = HPC * DH  # v shard features = 256
MID = MULT * D  # full mlp rows = 4096 (sequence-parallel MLP)
AF = mybir.ActivationFunctionType
ALU = mybir.AluOpType


def build_nc(T):
    """Build the SPMD graph for one core (token count T per batch)."""
    DC = D // 128  # d chunks = 8
    TT = T // 128  # token tiles = 16
    QT = 512  # q-tile width == RS chunk width
    NQ = T // QT  # 4
    CPQ = QT // 128  # 4
    MB = MID // 128  # mlp row blocks = 32
    NT = D // 512
    OW = T // TP  # owned tokens = 512
    OT = OW // 128  # owned token tiles = 4

    nc = bacc.Bacc("TRN2", target_bir_lowering=False, debug=False, num_devices=8)

    x_e = nc.dram_tensor("x", [T, D], F32, kind="ExternalInput")
    qkw_e = nc.dram_tensor("qkw_t", [D, QKF], BF16, kind="ExternalInput")
    vw_e = nc.dram_tensor("vw_m", [D, VF], BF16, kind="ExternalInput")
    ow_e = nc.dram_tensor("ow_m", [VF, D], BF16, kind="ExternalInput")
    w1w_e = nc.dram_tensor("w1t", [128, MB * D], BF16, kind="ExternalInput")
    wgw_e = nc.dram_tensor("wgt", [128, MB * D], BF16, kind="ExternalInput")
    w2w_e = nc.dram_tensor("w2t", [128, DC * MID], BF16, kind="ExternalInput")
    cos_e = nc.dram_tensor("cosr", [128, T], BF16, kind="ExternalInput")
    sin_e = nc.dram_tensor("sinr", [128, T], BF16, kind="ExternalInput")
    cm_e = nc.dram_tensor("cmask", [CPQ * 128, QT], BF16, kind="ExternalInput")
    id_e = nc.dram_tensor("ident", [128, 128], BF16, kind="ExternalInput")
    sel_e = nc.dram_tensor("selc", [256, 128], BF16, kind="ExternalInput")
    out_e = nc.dram_tensor("out", [OW, D], F32, kind="ExternalOutput")

    groups = [[0, 1, 2, 3], [4, 5, 6, 7]]

    with tile.TileContext(nc) as tc:
        with (
            tc.tile_pool(name="const", bufs=1) as cpool,
            tc.tile_pool(name="actfm", bufs=1) as fmpool,
            tc.tile_pool(name="qko", bufs=1) as qkpool,
            tc.tile_pool(name="vaug", bufs=1) as vpool,
            tc.tile_pool(name="frag", bufs=1) as fpool,
            tc.tile_pool(name="xin", bufs=2) as xpool,
            tc.tile_pool(name="xnb", bufs=3) as xnpool,
            tc.tile_pool(name="work", bufs=4) as wpool,
            tc.tile_pool(name="rope", bufs=2) as rpool,
            tc.tile_pool(name="stats", bufs=8) as spool,
            tc.tile_pool(name="psA", bufs=4, space="PSUM") as psA,
            tc.tile_pool(name="psO", bufs=2, space="PSUM") as psO,
            tc.tile_pool(name="psS", bufs=2, space="PSUM") as psS,
            tc.tile_pool(name="dram", bufs=1, space="DRAM") as dpool,
        ):
            # ---- resident weights / tables ----
            def load_tiles(src, width, n, dt=BF16):
                ts = []
                for i in range(n):
                    t = cpool.tile(
                        [128, width], dt, tag=f"{src.name}_{i}", name=f"{src.name}_{i}"
                    )
                    nc.sync.dma_start(t[:], src[i * 128 : (i + 1) * 128, :])
                    ts.append(t)
                return ts

            qkw = load_tiles(qkw_e, QKF, DC)
            vw = load_tiles(vw_e, VF, DC)
            ow = load_tiles(ow_e, D, VF // 128)
            cosr = load_tiles(cos_e, T, 1)[0]
            sinr = load_tiles(sin_e, T, 1)[0]
            nmask = load_tiles(cm_e, QT, CPQ)
            ident = load_tiles(id_e, 128, 1)[0]
            epsc = cpool.tile([128, 1], F32, tag="epsc", name="epsc")
            nc.vector.memset(epsc[:], EPS)
            # head-select tiles for denominator broadcast:
            # sel[ot][p, c] = 1 iff p == 32*(2*ot + c//64) (host-precomputed)
            sel = []
            for ot in range(2):
                s = cpool.tile([128, 128], BF16, tag=f"sel{ot}", name=f"sel{ot}")
                nc.sync.dma_start(s[:], sel_e[ot * 128 : (ot + 1) * 128, :])
                sel.append(s)

            rs_in = dpool.tile([T, D], BF16, name="rs_in")
            rs_out = dpool.tile([OW, D], BF16, name="rs_out")

            # warmup collective: pay first-CC setup cost during stage A
            wu_in = dpool.tile([TP, 16], BF16, name="wu_in")
            wu_out = dpool.tile([1, 16], BF16, name="wu_out")
            nc.gpsimd.collective_compute(
                "ReduceScatter",
                ALU.add,
                ins=[wu_in[:, :].opt()],
                outs=[wu_out[:, :].opt()],
                replica_groups=groups,
            )

            # ---- persistent activation tiles ----
            xnf = [
                fmpool.tile([128, T], BF16, tag=f"fm{d}", name=f"xnf{d}")
                for d in range(DC)
            ]
            q_sb = [
                qkpool.tile([128, T], BF16, tag=f"qk{i}", name=f"q{i}")
                for i in range(2)
            ]
            k_sb = [
                qkpool.tile([128, T], BF16, tag=f"qk{i + 2}", name=f"k{i}")
                for i in range(2)
            ]
            O_sb = [
                qkpool.tile([128, T], BF16, tag=f"qk{i + 4}", name=f"O{i}")
                for i in range(2)
            ]
            On_sb = [
                qkpool.tile([128, T], BF16, tag=f"qk{i + 6}", name=f"On{i}")
                for i in range(2)
            ]
            v_aug = [
                vpool.tile([128, HPC, DH + 1], BF16, tag=f"va{ti}", name=f"va{ti}")
                for ti in range(TT)
            ]
            # owned-token tiles (sequence-parallel MLP)
            hnf = [
                fpool.tile([128, OW], BF16, tag=f"hn{d}", name=f"hnf{d}")
                for d in range(DC)
            ]
            h1f = [
                fpool.tile([128, D], BF16, tag=f"h1f{k}", name=f"h1f{k}")
                for k in range(OT)
            ]
            out_sb = [
                fpool.tile([128, D], BF16, tag=f"os{k}", name=f"out_sb{k}")
                for k in range(OT)
            ]

            # ---- filler queue: independent work pumped into attn stalls ----
            fillers = []

            def pump():
                if fillers:
                    fillers.pop(0)()

            def drain():
                while fillers:
                    fillers.pop(0)()

            # ---- helpers ----
            def norm_into_fm(xt, fm_tiles, ti):
                """rmsnorm the token tile xt, write bf16 feature-major at col ti."""
                ss = spool.tile([128, 1], F32, tag="ss", name="ss")
                sq = xnpool.tile([128, D], BF16, tag="sq", name="sq", bufs=2)
                nc.scalar.activation(
                    out=sq[:], in_=xt[:], func=AF.Square, accum_out=ss[:]
                )
                sr = spool.tile([128, 1], F32, tag="sr", name="sr")
                nc.scalar.activation(
                    out=sr[:], in_=ss[:], func=AF.Sqrt, bias=epsc[:], scale=1.0 / D
                )
                s1 = spool.tile([128, 1], F32, tag="s1", name="s1")
                nc.vector.reciprocal(s1[:], sr[:])
                xn = xnpool.tile([128, D], BF16, tag="xn", name="xn")
                nc.vector.tensor_scalar_mul(xn[:], xt[:], s1[:])
                for di in range(DC):
                    tp = psS.tile([128, 128], BF16, tag="tp", name="tp", bufs=2)
                    nc.tensor.transpose(
                        tp[:], xn[:, di * 128 : (di + 1) * 128], ident[:]
                    )
                    nc.any.tensor_copy(
                        fm_tiles[di][:, ti * 128 : (ti + 1) * 128], tp[:]
                    )

            def norm_tile(ti):
                xt = xpool.tile([128, D], F32, tag="xt", name="xt")
                nc.sync.dma_start(xt[:], x_e[ti * 128 : (ti + 1) * 128, :])
                norm_into_fm(xt, xnf, ti)

            def qk_m(t4, m):  # one of q01 q23 k01 k23 for chunk t4
                tsl = slice(t4 * QT, (t4 + 1) * QT)
                dst = q_sb[m] if m < 2 else k_sb[m - 2]
                ps = psA.tile([128, 512], F32, tag="ps", name="ps")
                for dc in range(DC):
                    nc.tensor.matmul(
                        ps[:, :QT],
                        qkw[dc][:, m * 128 : (m + 1) * 128],
                        xnf[dc][:, tsl],
                        start=(dc == 0),
                        stop=(dc == DC - 1),
                    )
                qb = rpool.tile([128, QT], BF16, tag="qb", name="qb")
                nc.vector.tensor_copy(qb[:], ps[:, :QT])
                rot = rpool.tile([128, QT], BF16, tag="rot", name="rot")
                for hb in (0, 64):
                    nc.vector.tensor_scalar_mul(
                        rot[hb : hb + 32, :], qb[hb + 32 : hb + 64, :], -1.0
                    )
                    nc.vector.tensor_copy(
                        rot[hb + 32 : hb + 64, :], qb[hb : hb + 32, :]
                    )
                t1 = rpool.tile([128, QT], BF16, tag="t1", name="t1")
                nc.vector.tensor_mul(t1[:], qb[:], cosr[:, tsl])
                t2 = rpool.tile([128, QT], BF16, tag="t2", name="t2")
                nc.vector.tensor_mul(t2[:], rot[:], sinr[:, tsl])
                nc.vector.tensor_add(dst[:, tsl], t1[:], t2[:])

            def v_ti(ti):
                ps = psS.tile([128, VF], F32, tag="tp", name="psv")
                for dc in range(DC):
                    nc.tensor.matmul(
                        ps[:],
                        xnf[dc][:, ti * 128 : (ti + 1) * 128],
                        vw[dc][:],
                        start=(dc == 0),
                        stop=(dc == DC - 1),
                    )
                va = v_aug[ti]
                nc.vector.tensor_copy(
                    va[:, :, 0:DH], ps.rearrange("p (h d) -> p h d", h=HPC)
                )
                nc.vector.memset(va[:, :, DH : DH + 1], 1.0)

            # ---- attention ----
            def attn_qtile(qt):
                tsl = slice(qt * QT, (qt + 1) * QT)
                ncks = CPQ * (qt + 1)
                recf = spool.tile([128, QT], F32, tag="recf", name="recf", bufs=2)
                for hp in range(2):
                    opsP = [
                        psO.tile([DH + 1, QT], F32, tag="pso", name=f"ops{i}")
                        for i in range(2)
                    ]
                    for ck in range(ncks):
                        j = ck - CPQ * qt
                        pts = []
                        for i in range(2):
                            hb = i * 64
                            sp = psA.tile([128, 512], F32, tag="ps", name="sp")
                            nc.tensor.matmul(
                                sp[:, :QT],
                                k_sb[hp][hb : hb + DH, ck * 128 : (ck + 1) * 128],
                                q_sb[hp][hb : hb + DH, tsl],
                                start=True,
                                stop=(j < 0),
                            )
                            if j >= 0:  # fold causal mask into the psum
                                nc.tensor.matmul(
                                    sp[:, :QT],
                                    ident[:],
                                    nmask[j][:],
                                    start=False,
                                    stop=True,
                                )
                            pt = wpool.tile(
                                [128, QT], BF16, tag="pt", name="pt", bufs=4
                            )
                            nc.scalar.activation(
                                out=pt[:], in_=sp[:, :QT], func=AF.Exp, scale=0.125
                            )
                            pts.append(pt)
                        pump()
                        for i in range(2):
                            nc.tensor.matmul(
                                opsP[i][:],
                                v_aug[ck][:, 2 * hp + i, :],
                                pts[i][:],
                                start=(ck == 0),
                                stop=(ck == ncks - 1),
                            )
                    for i in range(2):
                        h = 2 * hp + i
                        nc.vector.tensor_copy(
                            recf[32 * h : 32 * h + 1, :], opsP[i][DH : DH + 1, :]
                        )
                        nc.scalar.copy(
                            O_sb[hp][i * 64 : i * 64 + DH, tsl], opsP[i][0:DH, :]
                        )
                rec32 = spool.tile([128, QT], F32, tag="rec32", name="rec32", bufs=2)
                nc.vector.reciprocal(rec32[:], recf[:])
                recb = spool.tile([128, QT], BF16, tag="recb", name="recb", bufs=2)
                nc.gpsimd.tensor_copy(recb[:], rec32[:])
                return recb

            def normalize_qt(qt, recb):
                tsl = slice(qt * QT, (qt + 1) * QT)
                for ot in range(2):
                    bb = psA.tile([128, 512], F32, tag="ps", name="bb")
                    nc.tensor.matmul(
                        bb[:, :QT], sel[ot][:], recb[:], start=True, stop=True
                    )
                    nc.vector.tensor_mul(
                        On_sb[ot][:, tsl], O_sb[ot][:, tsl], bb[:, :QT]
                    )

            def oproj_ti(ti):  # o-proj partial + x/TP for one 128-token tile
                ob = wpool.tile([128, D], BF16, tag="ob", name="ob", bufs=3)
                xo = xpool.tile([128, D], F32, tag="xo", name="xo")
                nc.sync.dma_start(xo[:], x_e[ti * 128 : (ti + 1) * 128, :])
                for nt in range(NT):
                    ps = psA.tile([128, 512], F32, tag="ps", name="ps")
                    for c in range(VF // 128):
                        nc.tensor.matmul(
                            ps[:, :512],
                            On_sb[c][:, ti * 128 : (ti + 1) * 128],
                            ow[c][:, nt * 512 : (nt + 1) * 512],
                            start=(c == 0),
                            stop=(c == VF // 128 - 1),
                        )
                    nc.vector.scalar_tensor_tensor(
                        ob[:, nt * 512 : (nt + 1) * 512],
                        xo[:, nt * 512 : (nt + 1) * 512],
                        1.0 / TP,
                        ps[:, :512],
                        ALU.mult,
                        ALU.add,
                    )
                nc.sync.dma_start(rs_in[ti * 128 : (ti + 1) * 128, :], ob[:])

            def rs_fire(qt):
                nc.gpsimd.collective_compute(
                    "ReduceScatter",
                    ALU.add,
                    ins=[rs_in[qt * QT : (qt + 1) * QT, :].opt()],
                    outs=[rs_out[qt * 128 : (qt + 1) * 128, :].opt()],
                    replica_groups=groups,
                )

            def resid_frag(qt):
                """h1 fragment qt (128 owned tokens) -> norm2 feature-major."""
                h1 = h1f[qt]
                nc.gpsimd.dma_start(h1[:], rs_out[qt * 128 : (qt + 1) * 128, :])
                norm_into_fm(h1, hnf, qt)

            # ---- MLP pieces (sequence-parallel, full weights) ----
            a_t = [
                fmpool.tile([128, T], BF16, tag=f"fm{j}", name=f"a{j}")
                for j in range(DC)
            ]

            def a_slice(mc, cols=slice(0, OW)):
                base = (mc % 4) * OW
                return a_t[mc // 4][:, base + cols.start : base + cols.stop]

            def mlp_block(mc, half):
                """wg/w1/silu/mul for row-block mc, token cols half*256..+256."""
                cols = slice(half * (OW // 2), (half + 1) * (OW // 2))
                CW = OW // 2
                wgb = wpool.tile([128, D], BF16, tag="wgs", name="wgb", bufs=3)
                nc.sync.dma_start(wgb[:], wgw_e[:, mc * D : (mc + 1) * D])
                w1b = wpool.tile([128, D], BF16, tag="w1s", name="w1b", bufs=3)
                nc.sync.dma_start(w1b[:], w1w_e[:, mc * D : (mc + 1) * D])
                psg = psA.tile([128, 512], F32, tag="ps", name="psg")
                for dc in range(DC):
                    nc.tensor.matmul(
                        psg[:, :CW],
                        wgb[:, dc * 128 : (dc + 1) * 128],
                        hnf[dc][:, cols],
                        start=(dc == 0),
                        stop=(dc == DC - 1),
                    )
                g_sb = wpool.tile([128, CW], BF16, tag="g", name="g", bufs=2)
                nc.scalar.activation(out=g_sb[:], in_=psg[:, :CW], func=AF.Silu)
                psu = psA.tile([128, 512], F32, tag="ps", name="psu")
                for dc in range(DC):
                    nc.tensor.matmul(
                        psu[:, :CW],
                        w1b[:, dc * 128 : (dc + 1) * 128],
                        hnf[dc][:, cols],
                        start=(dc == 0),
                        stop=(dc == DC - 1),
                    )
                nc.vector.tensor_mul(a_slice(mc, cols), g_sb[:], psu[:, :CW])

            # ---- schedule ----
            # front: chunk 0 norm + qkv
            for ti in range(CPQ):
                norm_tile(ti)
            for m in range(4):
                qk_m(0, m)
            for ti in range(CPQ):
                v_ti(ti)

            for qt in range(NQ):
                nxt = qt + 1
                if nxt < NQ:
                    for ti in range(nxt * CPQ, (nxt + 1) * CPQ):
                        fillers.append(lambda ti=ti: norm_tile(ti))
                    for m in range(4):
                        fillers.append(lambda m=m, nxt=nxt: qk_m(nxt, m))
                    for ti in range(nxt * CPQ, (nxt + 1) * CPQ):
                        fillers.append(lambda ti=ti: v_ti(ti))
                if qt >= 1:
                    prv = qt - 1
                    for ti in range(prv * CPQ, (prv + 1) * CPQ):
                        fillers.append(lambda ti=ti: oproj_ti(ti))
                    fillers.append(lambda prv=prv: rs_fire(prv))
                if qt == 3:
                    fillers.append(lambda: resid_frag(0))
                    fillers.append(lambda: resid_frag(1))
                    # first MLP half-pass (token cols 0:256 <- frags 0,1)
                    for mc in range(MB):
                        fillers.append(lambda mc=mc: mlp_block(mc, 0))
                recb = attn_qtile(qt)
                normalize_qt(qt, recb)
                if qt < 3:
                    drain()

            # critical path first: last o-proj chunk + its RS
            for ti in range((NQ - 1) * CPQ, NQ * CPQ):
                oproj_ti(ti)
            rs_fire(NQ - 1)
            drain()  # remaining first-half MLP blocks
            resid_frag(2)
            resid_frag(3)
            for mc in range(MB):  # second half-pass (cols 256:512 <- frags 2,3)
                mlp_block(mc, 1)

            # w2: feature-major out, then PE-transpose back to token-major.
            # Transposes for block `do` are emitted after the po-accumulation
            # of block do+1 so the PE never waits on the scalar pob copy.
            pend = []

            def w2_flush():
                while pend:
                    pob, do = pend.pop(0)
                    for tt in range(OT):
                        tp = psS.tile([128, 128], BF16, tag="tp", name="tp", bufs=2)
                        nc.tensor.transpose(
                            tp[:], pob[:, tt * 128 : (tt + 1) * 128], ident[:]
                        )
                        nc.any.tensor_copy(
                            out_sb[tt][:, do * 128 : (do + 1) * 128], tp[:]
                        )

            for do in range(DC):
                po = psA.tile([128, 512], F32, tag="ps", name="po")
                for half in range(2):
                    w2b = wpool.tile(
                        [128, MID // 2], BF16, tag="w2s", name="w2b", bufs=2
                    )
                    nc.sync.dma_start(
                        w2b[:],
                        w2w_e[
                            :,
                            do * MID + half * MID // 2 : do * MID + (half + 1) * MID // 2,
                        ],
                    )
                    for jj in range(MB // 2):
                        mc = half * MB // 2 + jj
                        nc.tensor.matmul(
                            po[:, :OW],
                            w2b[:, jj * 128 : (jj + 1) * 128],
                            a_slice(mc),
                            start=(mc == 0),
                            stop=(mc == MB - 1),
                        )
                pob = wpool.tile([128, OW], BF16, tag="pob", name="pob", bufs=3)
                nc.scalar.copy(pob[:], po[:, :OW])
                pend.append((pob, do))
                if do >= 1:
                    w2_flush()
            w2_flush()

            for tt in range(OT):
                ot = xpool.tile([128, D], F32, tag="xt", name="ot")
                nc.vector.tensor_add(ot[:], out_sb[tt][:], h1f[tt][:])
                nc.sync.dma_start(out_e[tt * 128 : (tt + 1) * 128, :], ot[:])

    nc.compile()
    return nc


def make_in_maps(x, n1_w, n2_w, qkv_w, o_w, w1_w, wg_w, w2_w, T):
    QT = 512
    CPQ = QT // 128
    half = DH // 2
    freqs = np.arange(half, dtype=np.float64) / half
    theta = 1.0 / ROPE_BASE**freqs
    ang = np.arange(T, dtype=np.float64)[:, None] * theta[None, :]  # [T, 32]
    p = np.arange(128) % half
    cosr = np.cos(ang)[:, p].T.astype(BF)  # [128, T]
    sinr = np.sin(ang)[:, p].T.astype(BF)
    # additive causal mask: 0 where allowed, -800 where masked
    cm = np.zeros((CPQ * 128, QT), dtype=BF)
    for j in range(CPQ):
        tk = np.arange(128)[:, None]
        tq = np.arange(QT)[None, :]
        cm[j * 128 : (j + 1) * 128] = np.where(tq >= j * 128 + tk, 0.0, -800.0).astype(
            BF
        )

    # head-select const: selc[ot*128 + p, c] = 1 iff p == 32*(2*ot + c//64)
    selc = np.zeros((256, 128), dtype=BF)
    for ot in range(2):
        for h in range(2):
            selc[ot * 128 + 32 * (2 * ot + h), h * 64 : (h + 1) * 64] = 1.0

    # full MLP weights, tiled for contiguous [128, D] / [128, MID] streams
    MB = MULT * D // 128  # 32
    DCn = D // 128  # 8
    Wg = (wg_w * n2_w[None, :]).T.astype(BF)  # [D, 4096]
    W1 = (w1_w * n2_w[None, :]).T.astype(BF)
    # tile[p, mc, c*128+j] = W[c*128+p, mc*128+j]
    wgt = Wg.reshape(DCn, 128, MB, 128).transpose(1, 2, 0, 3).reshape(128, MB * D)
    w1t = W1.reshape(DCn, 128, MB, 128).transpose(1, 2, 0, 3).reshape(128, MB * D)
    W2 = w2_w.T.astype(BF)  # [4096, D]
    # tile[p, do, mc, j] = W2[mc*128+p, do*128+j]
    w2t = (
        W2.reshape(MB, 128, DCn, 128)
        .transpose(1, 2, 0, 3)
        .reshape(128, DCn * MB * 128)
    )

    in_maps = []
    for c in range(8):
        b, r = c // 4, c % 4
        qs = slice(r * VF, (r + 1) * VF)
        qr = qkv_w[0 * D :][qs] * n1_w[None, :]
        kr = qkv_w[1 * D :][qs] * n1_w[None, :]
        vr = qkv_w[2 * D :][qs] * n1_w[None, :]
        in_maps.append(
            {
                "x": np.ascontiguousarray(x[b, :T], np.float32),
                "qkw_t": np.ascontiguousarray(
                    np.concatenate([qr, kr], 0).T.astype(BF)
                ),
                "vw_m": np.ascontiguousarray(vr.T.astype(BF)),
                "ow_m": np.ascontiguousarray(o_w[:, qs].T.astype(BF)),
                "w1t": np.ascontiguousarray(w1t),
                "wgt": np.ascontiguousarray(wgt),
                "w2t": np.ascontiguousarray(w2t),
                "cosr": cosr,
                "sinr": sinr,
                "cmask": cm,
                "ident": np.eye(128, dtype=BF),
                "selc": selc,
            }
        )
    return in_maps


_CACHE = {}


def _get_nc(T):
    if T not in _CACHE:
        _CACHE[T] = build_nc(T)
    return _CACHE[T]


def run(inputs, T=2048, trace=False):
    nc = _get_nc(T)
    in_maps = make_in_maps(T=T, **inputs)
    res = run_bass_kernel_spmd(nc, in_maps, core_ids=list(range(8)), trace=trace)
    out = np.empty((B, T, D), np.float32)
    QT = 512
    for c in range(8):
        b, r = c // 4, c % 4
        o = res.results[c]["out"]  # [T//TP, D]: fragment qt at rows qt*128
        for qt in range(T // QT):
            out[b, qt * QT + r * 128 : qt * QT + (r + 1) * 128] = o[
                qt * 128 : (qt + 1) * 128
            ]
    return out, res


def kernel(**inputs):
    out, _ = run(inputs, T=2048)
    return out


# revision 20
# speedup vs baseline: 1.0291x; 1.0291x over previous
"""Distributed Trainium2 kernel for a dense transformer block.

Reference computation (per batch):
  x = x + o_proj(attn(rope(qkv(rmsnorm(x))), causal)) ; x = x + w2(silu(wg(rmsnorm(x))) * w1(rmsnorm(x)))

Sharding: DP=2 on batch x 4-way hybrid within each batch group.
Cores 0-3 handle batch 0, cores 4-7 batch 1. Attention is tensor-parallel
(rank r owns heads 4r..4r+3); a chunked ReduceScatter then hands each rank
ownership of 512 tokens (4 fragments of 128), and the MLP runs
sequence-parallel with full weights streamed from HBM — no second
collective and no final AllGather (the host reassembles output slices).

The PE clock ramps with continuous busy time (0.65 -> 1.2 -> 2.4 GHz), so
independent matmul work (next chunk's qkv, previous chunk's o-proj, first
MLP half-pass) is interleaved into the attention chain's stall points via
a filler queue to keep the tensor engine saturated.
"""

import sys

sys.path.insert(0, "/opt/trn_rl_repo")

import numpy as np
import ml_dtypes

import concourse.bass as bass
import concourse.bacc as bacc
import concourse.mybir as mybir
import concourse.tile as tile
from concourse.bass_utils import run_bass_kernel_spmd

BF = ml_dtypes.bfloat16
F32 = mybir.dt.float32
BF16 = mybir.dt.bfloat16

D = 1024
NH = 16
DH = 64
MULT = 4
EPS = 1e-5
ROPE_BASE = 10000.0
B = 2
TP = 4  # ranks per group
HPC = NH // TP  # heads per core = 4
QKF = 2 * HPC * DH  # q+k shard features = 512
VF = HPC * DH  # v shard features = 256
MID = MULT * D  # full mlp rows = 4096 (sequence-parallel MLP)
AF = mybir.ActivationFunctionType
ALU = mybir.AluOpType


def build_nc(T):
    """Build the SPMD graph for one core (token count T per batch)."""
    DC = D // 128  # d chunks = 8
    TT = T // 128  # token tiles = 16
    QT = 512  # q-tile width == RS chunk width
    NQ = T // QT  # 4
    CPQ = QT // 128  # 4
    MB = MID // 128  # mlp row blocks = 32
    NT = D // 512
    OW = T // TP  # owned tokens = 512
    OT = OW // 128  # owned token tiles = 4

    nc = bacc.Bacc("TRN2", target_bir_lowering=False, debug=False, num_devices=8)

    x_e = nc.dram_tensor("x", [T, D], F32, kind="ExternalInput")
    qkw_e = nc.dram_tensor("qkw_t", [D, QKF], BF16, kind="ExternalInput")
    vw_e = nc.dram_tensor("vw_m", [D, VF], BF16, kind="ExternalInput")
    ow_e = nc.dram_tensor("ow_m", [VF, D], BF16, kind="ExternalInput")
    w1w_e = nc.dram_tensor("w1t", [128, MB * D], BF16, kind="ExternalInput")
    wgw_e = nc.dram_tensor("wgt", [128, MB * D], BF16, kind="ExternalInput")
    w2w_e = nc.dram_tensor("w2t", [128, DC * MID], BF16, kind="ExternalInput")
    cos_e = nc.dram_tensor("cosr", [128, T], BF16, kind="ExternalInput")
    sin_e = nc.dram_tensor("sinr", [128, T], BF16, kind="ExternalInput")
    cm_e = nc.dram_tensor("cmask", [CPQ * 128, QT], BF16, kind="ExternalInput")
    id_e = nc.dram_tensor("ident", [128, 128], BF16, kind="ExternalInput")
    sel_e = nc.dram_tensor("selc", [256, 128], BF16, kind="ExternalInput")
    out_e = nc.dram_tensor("out", [OW, D], F32, kind="ExternalOutput")

    groups = [[0, 1, 2, 3], [4, 5, 6, 7]]

    with tile.TileContext(nc) as tc:
        with (
            tc.tile_pool(name="const", bufs=1) as cpool,
            tc.tile_pool(name="actfm", bufs=1) as fmpool,
            tc.tile_pool(name="qko", bufs=1) as qkpool,
            tc.tile_pool(name="vaug", bufs=1) as vpool,
            tc.tile_pool(name="frag", bufs=1) as fpool,
            tc.tile_pool(name="xin", bufs=2) as xpool,
            tc.tile_pool(name="xnb", bufs=3) as xnpool,
            tc.tile_pool(name="work", bufs=4) as wpool,
            tc.tile_pool(name="rope", bufs=2) as rpool,
            tc.tile_pool(name="stats", bufs=8) as spool,
            tc.tile_pool(name="psA", bufs=4, space="PSUM") as psA,
            tc.tile_pool(name="psO", bufs=2, space="PSUM") as psO,
            tc.tile_pool(name="psS", bufs=2, space="PSUM") as psS,
            tc.tile_pool(name="dram", bufs=1, space="DRAM") as dpool,
        ):
            # ---- resident weights / tables ----
            def load_tiles(src, width, n, dt=BF16):
                ts = []
                for i in range(n):
                    t = cpool.tile(
                        [128, width], dt, tag=f"{src.name}_{i}", name=f"{src.name}_{i}"
                    )
                    nc.sync.dma_start(t[:], src[i * 128 : (i + 1) * 128, :])
                    ts.append(t)
                return ts

            qkw = load_tiles(qkw_e, QKF, DC)
            vw = load_tiles(vw_e, VF, DC)
            ow = load_tiles(ow_e, D, VF // 128)
            cosr = load_tiles(cos_e, T, 1)[0]
            sinr = load_tiles(sin_e, T, 1)[0]
            nmask = load_tiles(cm_e, QT, CPQ)
            ident = load_tiles(id_e, 128, 1)[0]
            epsc = cpool.tile([128, 1], F32, tag="epsc", name="epsc")
            nc.vector.memset(epsc[:], EPS)
            # head-select tiles for denominator broadcast:
            # sel[ot][p, c] = 1 iff p == 32*(2*ot + c//64) (host-precomputed)
            sel = []
            for ot in range(2):
                s = cpool.tile([128, 128], BF16, tag=f"sel{ot}", name=f"sel{ot}")
                nc.sync.dma_start(s[:], sel_e[ot * 128 : (ot + 1) * 128, :])
                sel.append(s)

            rs_in = dpool.tile([T, D], BF16, name="rs_in")
            rs_out = dpool.tile([OW, D], BF16, name="rs_out")

            # warmup collective: full-size dummy RS so the first real chunk
            # doesn't pay the large-transfer CC setup cost
            wu_in = dpool.tile([QT, D], BF16, name="wu_in")
            wu_out = dpool.tile([QT // TP, D], BF16, name="wu_out")
            nc.gpsimd.collective_compute(
                "ReduceScatter",
                ALU.add,
                ins=[wu_in[:, :].opt()],
                outs=[wu_out[:, :].opt()],
                replica_groups=groups,
            )

            # ---- persistent activation tiles ----
            xnf = [
                fmpool.tile([128, T], BF16, tag=f"fm{d}", name=f"xnf{d}")
                for d in range(DC)
            ]
            q_sb = [
                qkpool.tile([128, T], BF16, tag=f"qk{i}", name=f"q{i}")
                for i in range(2)
            ]
            k_sb = [
                qkpool.tile([128, T], BF16, tag=f"qk{i + 2}", name=f"k{i}")
                for i in range(2)
            ]
            O_sb = [
                qkpool.tile([128, T], BF16, tag=f"qk{i + 4}", name=f"O{i}")
                for i in range(2)
            ]
            On_sb = [
                qkpool.tile([128, T], BF16, tag=f"qk{i + 6}", name=f"On{i}")
                for i in range(2)
            ]
            v_aug = [
                vpool.tile([128, HPC, DH + 1], BF16, tag=f"va{ti}", name=f"va{ti}")
                for ti in range(TT)
            ]
            # owned-token tiles (sequence-parallel MLP)
            hnf = [
                fpool.tile([128, OW], BF16, tag=f"hn{d}", name=f"hnf{d}")
                for d in range(DC)
            ]
            h1f = [
                fpool.tile([128, D], BF16, tag=f"h1f{k}", name=f"h1f{k}")
                for k in range(OT)
            ]
            out_sb = [
                fpool.tile([128, D], BF16, tag=f"os{k}", name=f"out_sb{k}")
                for k in range(OT)
            ]

            # ---- filler queue: independent work pumped into attn stalls ----
            fillers = []

            def pump():
                if fillers:
                    fillers.pop(0)()

            def drain():
                while fillers:
                    fillers.pop(0)()

            # ---- helpers ----
            def norm_into_fm(xt, fm_tiles, ti):
                """rmsnorm the token tile xt, write bf16 feature-major at col ti."""
                ss = spool.tile([128, 1], F32, tag="ss", name="ss")
                sq = xnpool.tile([128, D], BF16, tag="sq", name="sq", bufs=2)
                nc.scalar.activation(
                    out=sq[:], in_=xt[:], func=AF.Square, accum_out=ss[:]
                )
                sr = spool.tile([128, 1], F32, tag="sr", name="sr")
                nc.scalar.activation(
                    out=sr[:], in_=ss[:], func=AF.Sqrt, bias=epsc[:], scale=1.0 / D
                )
                s1 = spool.tile([128, 1], F32, tag="s1", name="s1")
                nc.vector.reciprocal(s1[:], sr[:])
                xn = xnpool.tile([128, D], BF16, tag="xn", name="xn")
                nc.vector.tensor_scalar_mul(xn[:], xt[:], s1[:])
                for di in range(DC):
                    tp = psS.tile([128, 128], BF16, tag="tp", name="tp", bufs=2)
                    nc.tensor.transpose(
                        tp[:], xn[:, di * 128 : (di + 1) * 128], ident[:]
                    )
                    nc.any.tensor_copy(
                        fm_tiles[di][:, ti * 128 : (ti + 1) * 128], tp[:]
                    )

            def norm_tile(ti):
                xt = xpool.tile([128, D], F32, tag="xt", name="xt")
                nc.sync.dma_start(xt[:], x_e[ti * 128 : (ti + 1) * 128, :])
                norm_into_fm(xt, xnf, ti)

            def qk_m(t4, m):  # one of q01 q23 k01 k23 for chunk t4
                tsl = slice(t4 * QT, (t4 + 1) * QT)
                dst = q_sb[m] if m < 2 else k_sb[m - 2]
                ps = psA.tile([128, 512], F32, tag="ps", name="ps")
                for dc in range(DC):
                    nc.tensor.matmul(
                        ps[:, :QT],
                        qkw[dc][:, m * 128 : (m + 1) * 128],
                        xnf[dc][:, tsl],
                        start=(dc == 0),
                        stop=(dc == DC - 1),
                    )
                qb = rpool.tile([128, QT], BF16, tag="qb", name="qb")
                nc.vector.tensor_copy(qb[:], ps[:, :QT])
                rot = rpool.tile([128, QT], BF16, tag="rot", name="rot")
                for hb in (0, 64):
                    nc.vector.tensor_scalar_mul(
                        rot[hb : hb + 32, :], qb[hb + 32 : hb + 64, :], -1.0
                    )
                    nc.vector.tensor_copy(
                        rot[hb + 32 : hb + 64, :], qb[hb : hb + 32, :]
                    )
                t1 = rpool.tile([128, QT], BF16, tag="t1", name="t1")
                nc.vector.tensor_mul(t1[:], qb[:], cosr[:, tsl])
                t2 = rpool.tile([128, QT], BF16, tag="t2", name="t2")
                nc.vector.tensor_mul(t2[:], rot[:], sinr[:, tsl])
                nc.vector.tensor_add(dst[:, tsl], t1[:], t2[:])

            def v_ti(ti):
                ps = psS.tile([128, VF], F32, tag="tp", name="psv")
                for dc in range(DC):
                    nc.tensor.matmul(
                        ps[:],
                        xnf[dc][:, ti * 128 : (ti + 1) * 128],
                        vw[dc][:],
                        start=(dc == 0),
                        stop=(dc == DC - 1),
                    )
                va = v_aug[ti]
                nc.vector.tensor_copy(
                    va[:, :, 0:DH], ps.rearrange("p (h d) -> p h d", h=HPC)
                )
                nc.vector.memset(va[:, :, DH : DH + 1], 1.0)

            # ---- attention ----
            def attn_qtile(qt):
                tsl = slice(qt * QT, (qt + 1) * QT)
                ncks = CPQ * (qt + 1)
                recf = spool.tile([128, QT], F32, tag="recf", name="recf", bufs=2)
                nc.vector.memset(recf[:], 1.0)
                for hp in range(2):
                    opsP = [
                        psO.tile([DH + 1, QT], F32, tag="pso", name=f"ops{i}")
                        for i in range(2)
                    ]
                    for ck in range(ncks):
                        j = ck - CPQ * qt
                        pts = []
                        for i in range(2):
                            hb = i * 64
                            sp = psA.tile([128, 512], F32, tag="ps", name="sp")
                            nc.tensor.matmul(
                                sp[:, :QT],
                                k_sb[hp][hb : hb + DH, ck * 128 : (ck + 1) * 128],
                                q_sb[hp][hb : hb + DH, tsl],
                                start=True,
                                stop=(j < 0),
                            )
                            if j >= 0:  # fold causal mask into the psum
                                nc.tensor.matmul(
                                    sp[:, :QT],
                                    ident[:],
                                    nmask[j][:],
                                    start=False,
                                    stop=True,
                                )
                            pt = wpool.tile(
                                [128, QT], BF16, tag="pt", name="pt", bufs=4
                            )
                            nc.scalar.activation(
                                out=pt[:], in_=sp[:, :QT], func=AF.Exp, scale=0.125
                            )
                            pts.append(pt)
                        pump()
                        for i in range(2):
                            nc.tensor.matmul(
                                opsP[i][:],
                                v_aug[ck][:, 2 * hp + i, :],
                                pts[i][:],
                                start=(ck == 0),
                                stop=(ck == ncks - 1),
                            )
                    for i in range(2):
                        h = 2 * hp + i
                        nc.vector.tensor_copy(
                            recf[32 * h : 32 * h + 1, :], opsP[i][DH : DH + 1, :]
                        )
                        nc.scalar.copy(
                            O_sb[hp][i * 64 : i * 64 + DH, tsl], opsP[i][0:DH, :]
                        )
                rec32 = spool.tile([128, QT], F32, tag="rec32", name="rec32", bufs=2)
                nc.vector.reciprocal(rec32[:], recf[:])
                recb = spool.tile([128, QT], BF16, tag="recb", name="recb", bufs=2)
                nc.gpsimd.tensor_copy(recb[:], rec32[:])
                return recb

            def normalize_qt(qt, recb):
                tsl = slice(qt * QT, (qt + 1) * QT)
                for ot in range(2):
                    bb = psA.tile([128, 512], F32, tag="ps", name="bb")
                    nc.tensor.matmul(
                        bb[:, :QT], sel[ot][:], recb[:], start=True, stop=True
                    )
                    nc.vector.tensor_mul(
                        On_sb[ot][:, tsl], O_sb[ot][:, tsl], bb[:, :QT]
                    )

            def oproj_ti(ti):  # o-proj partial + x/TP for one 128-token tile
                ob = wpool.tile([128, D], BF16, tag="ob", name="ob", bufs=3)
                xo = xpool.tile([128, D], F32, tag="xo", name="xo")
                nc.sync.dma_start(xo[:], x_e[ti * 128 : (ti + 1) * 128, :])
                for nt in range(NT):
                    ps = psA.tile([128, 512], F32, tag="ps", name="ps")
                    for c in range(VF // 128):
                        nc.tensor.matmul(
                            ps[:, :512],
                            On_sb[c][:, ti * 128 : (ti + 1) * 128],
                            ow[c][:, nt * 512 : (nt + 1) * 512],
                            start=(c == 0),
                            stop=(c == VF // 128 - 1),
                        )
                    nc.vector.scalar_tensor_tensor(
                        ob[:, nt * 512 : (nt + 1) * 512],
                        xo[:, nt * 512 : (nt + 1) * 512],
                        1.0 / TP,
                        ps[:, :512],
                        ALU.mult,
                        ALU.add,
                    )
                nc.sync.dma_start(rs_in[ti * 128 : (ti + 1) * 128, :], ob[:])

            def rs_fire(qt):
                nc.gpsimd.collective_compute(
                    "ReduceScatter",
                    ALU.add,
                    ins=[rs_in[qt * QT : (qt + 1) * QT, :].opt()],
                    outs=[rs_out[qt * 128 : (qt + 1) * 128, :].opt()],
                    replica_groups=groups,
                )

            def resid_frag(qt):
                """h1 fragment qt (128 owned tokens) -> norm2 feature-major."""
                h1 = h1f[qt]
                nc.gpsimd.dma_start(h1[:], rs_out[qt * 128 : (qt + 1) * 128, :])
                norm_into_fm(h1, hnf, qt)

            # ---- MLP pieces (sequence-parallel, full weights) ----
            a_t = [
                fmpool.tile([128, T], BF16, tag=f"fm{j}", name=f"a{j}")
                for j in range(DC)
            ]

            def a_slice(mc, cols=slice(0, OW)):
                base = (mc % 4) * OW
                return a_t[mc // 4][:, base + cols.start : base + cols.stop]

            def mlp_block(mc, half):
                """wg/w1/silu/mul for row-block mc, token cols half*256..+256."""
                cols = slice(half * (OW // 2), (half + 1) * (OW // 2))
                CW = OW // 2
                wgb = wpool.tile([128, D], BF16, tag="wgs", name="wgb", bufs=3)
                nc.sync.dma_start(wgb[:], wgw_e[:, mc * D : (mc + 1) * D])
                w1b = wpool.tile([128, D], BF16, tag="w1s", name="w1b", bufs=3)
                nc.sync.dma_start(w1b[:], w1w_e[:, mc * D : (mc + 1) * D])
                psg = psA.tile([128, 512], F32, tag="ps", name="psg")
                for dc in range(DC):
                    nc.tensor.matmul(
                        psg[:, :CW],
                        wgb[:, dc * 128 : (dc + 1) * 128],
                        hnf[dc][:, cols],
                        start=(dc == 0),
                        stop=(dc == DC - 1),
                    )
                g_sb = wpool.tile([128, CW], BF16, tag="g", name="g", bufs=2)
                nc.scalar.activation(out=g_sb[:], in_=psg[:, :CW], func=AF.Silu)
                psu = psA.tile([128, 512], F32, tag="ps", name="psu")
                for dc in range(DC):
                    nc.tensor.matmul(
                        psu[:, :CW],
                        w1b[:, dc * 128 : (dc + 1) * 128],
                        hnf[dc][:, cols],
                        start=(dc == 0),
                        stop=(dc == DC - 1),
                    )
                nc.vector.tensor_mul(a_slice(mc, cols), g_sb[:], psu[:, :CW])

            # ---- schedule ----
            # front: chunk 0 norm + qkv
            for ti in range(CPQ):
                norm_tile(ti)
            for m in range(4):
                qk_m(0, m)
            for ti in range(CPQ):
                v_ti(ti)

            for qt in range(NQ):
                nxt = qt + 1
                if nxt < NQ:
                    for ti in range(nxt * CPQ, (nxt + 1) * CPQ):
                        fillers.append(lambda ti=ti: norm_tile(ti))
                    for m in range(4):
                        fillers.append(lambda m=m, nxt=nxt: qk_m(nxt, m))
                    for ti in range(nxt * CPQ, (nxt + 1) * CPQ):
                        fillers.append(lambda ti=ti: v_ti(ti))
                if qt == 2:
                    fillers.append(lambda: resid_frag(0))
                if qt == 3:
                    fillers.append(lambda: resid_frag(1))
                    # first MLP half-pass (token cols 0:256 <- frags 0,1)
                    for mc in range(MB):
                        fillers.append(lambda mc=mc: mlp_block(mc, 0))
                recb = attn_qtile(qt)
                normalize_qt(qt, recb)
                # o-proj + RS on the critical path: fire the chunk ASAP
                for ti in range(qt * CPQ, (qt + 1) * CPQ):
                    oproj_ti(ti)
                rs_fire(qt)
                if qt < 3:
                    drain()

            drain()  # remaining first-half MLP blocks
            resid_frag(2)
            resid_frag(3)
            for mc in range(MB):  # second half-pass (cols 256:512 <- frags 2,3)
                mlp_block(mc, 1)

            # w2: feature-major out, then PE-transpose back to token-major.
            # Transposes for block `do` are emitted after the po-accumulation
            # of block do+1 so the PE never waits on the scalar pob copy.
            pend = []

            def w2_flush():
                while pend:
                    pob, do = pend.pop(0)
                    for tt in range(OT):
                        tp = psS.tile([128, 128], BF16, tag="tp", name="tp", bufs=2)
                        nc.tensor.transpose(
                            tp[:], pob[:, tt * 128 : (tt + 1) * 128], ident[:]
                        )
                        nc.any.tensor_copy(
                            out_sb[tt][:, do * 128 : (do + 1) * 128], tp[:]
                        )

            for do in range(DC):
                po = psA.tile([128, 512], F32, tag="ps", name="po")
                for half in range(2):
                    w2b = wpool.tile(
                        [128, MID // 2], BF16, tag="w2s", name="w2b", bufs=2
                    )
                    nc.sync.dma_start(
                        w2b[:],
                        w2w_e[
                            :,
                            do * MID + half * MID // 2 : do * MID + (half + 1) * MID // 2,
                        ],
                    )
                    for jj in range(MB // 2):
                        mc = half * MB // 2 + jj
                        nc.tensor.matmul(
                            po[:, :OW],
                            w2b[:, jj * 128 : (jj + 1) * 128],
                            a_slice(mc),
                            start=(mc == 0),
                            stop=(mc == MB - 1),
                        )
                pob = wpool.tile([128, OW], BF16, tag="pob", name="pob", bufs=3)
                nc.scalar.copy(pob[:], po[:, :OW])
                pend.append((pob, do))
                if do >= 1:
                    w2_flush()
            w2_flush()

            for tt in range(OT):
                ot = xpool.tile([128, D], F32, tag="xt", name="ot")
                nc.vector.tensor_add(ot[:], out_sb[tt][:], h1f[tt][:])
                nc.sync.dma_start(out_e[tt * 128 : (tt + 1) * 128, :], ot[:])

    nc.compile()
    return nc


def make_in_maps(x, n1_w, n2_w, qkv_w, o_w, w1_w, wg_w, w2_w, T):
    QT = 512
    CPQ = QT // 128
    half = DH // 2
    freqs = np.arange(half, dtype=np.float64) / half
    theta = 1.0 / ROPE_BASE**freqs
    ang = np.arange(T, dtype=np.float64)[:, None] * theta[None, :]  # [T, 32]
    p = np.arange(128) % half
    cosr = np.cos(ang)[:, p].T.astype(BF)  # [128, T]
    sinr = np.sin(ang)[:, p].T.astype(BF)
    # additive causal mask: 0 where allowed, -800 where masked
    cm = np.zeros((CPQ * 128, QT), dtype=BF)
    for j in range(CPQ):
        tk = np.arange(128)[:, None]
        tq = np.arange(QT)[None, :]
        cm[j * 128 : (j + 1) * 128] = np.where(tq >= j * 128 + tk, 0.0, -800.0).astype(
            BF
        )

    # head-select const: selc[ot*128 + p, c] = 1 iff p == 32*(2*ot + c//64)
    selc = np.zeros((256, 128), dtype=BF)
    for ot in range(2):
        for h in range(2):
            selc[ot * 128 + 32 * (2 * ot + h), h * 64 : (h + 1) * 64] = 1.0

    # full MLP weights, tiled for contiguous [128, D] / [128, MID] streams
    MB = MULT * D // 128  # 32
    DCn = D // 128  # 8
    Wg = (wg_w * n2_w[None, :]).T.astype(BF)  # [D, 4096]
    W1 = (w1_w * n2_w[None, :]).T.astype(BF)
    # tile[p, mc, c*128+j] = W[c*128+p, mc*128+j]
    wgt = Wg.reshape(DCn, 128, MB, 128).transpose(1, 2, 0, 3).reshape(128, MB * D)
    w1t = W1.reshape(DCn, 128, MB, 128).transpose(1, 2, 0, 3).reshape(128, MB * D)
    W2 = w2_w.T.astype(BF)  # [4096, D]
    # tile[p, do, mc, j] = W2[mc*128+p, do*128+j]
    w2t = (
        W2.reshape(MB, 128, DCn, 128)
        .transpose(1, 2, 0, 3)
        .reshape(128, DCn * MB * 128)
    )

    in_maps = []
    for c in range(8):
        b, r = c // 4, c % 4
        qs = slice(r * VF, (r + 1) * VF)
        qr = qkv_w[0 * D :][qs] * n1_w[None, :]
        kr = qkv_w[1 * D :][qs] * n1_w[None, :]
        vr = qkv_w[2 * D :][qs] * n1_w[None, :]
        in_maps.append(
            {
                "x": np.ascontiguousarray(x[b, :T], np.float32),
                "qkw_t": np.ascontiguousarray(
                    np.concatenate([qr, kr], 0).T.astype(BF)
                ),
                "vw_m": np.ascontiguousarray(vr.T.astype(BF)),
                "ow_m": np.ascontiguousarray(o_w[:, qs].T.astype(BF)),
                "w1t": np.ascontiguousarray(w1t),
                "wgt": np.ascontiguousarray(wgt),
                "w2t": np.ascontiguousarray(w2t),
                "cosr": cosr,
                "sinr": sinr,
                "cmask": cm,
                "ident": np.eye(128, dtype=BF),
                "selc": selc,
            }
        )
    return in_maps


_CACHE = {}


def _get_nc(T):
    if T not in _CACHE:
        _CACHE[T] = build_nc(T)
    return _CACHE[T]


def run(inputs, T=2048, trace=False):
    nc = _get_nc(T)
    in_maps = make_in_maps(T=T, **inputs)
    res = run_bass_kernel_spmd(nc, in_maps, core_ids=list(range(8)), trace=trace)
    out = np.empty((B, T, D), np.float32)
    QT = 512
    for c in range(8):
        b, r = c // 4, c % 4
        o = res.results[c]["out"]  # [T//TP, D]: fragment qt at rows qt*128
        for qt in range(T // QT):
            out[b, qt * QT + r * 128 : qt * QT + (r + 1) * 128] = o[
                qt * 128 : (qt + 1) * 128
            ]
    return out, res


def kernel(**inputs):
    out, _ = run(inputs, T=2048)
    return out


# revision 21
# speedup vs baseline: 1.1175x; 1.0859x over previous
"""Distributed Trainium2 kernel for a dense transformer block.

Reference computation (per batch):
  x = x + o_proj(attn(rope(qkv(rmsnorm(x))), causal)) ; x = x + w2(silu(wg(rmsnorm(x))) * w1(rmsnorm(x)))

Sharding: DP=2 on batch x 4-way hybrid within each batch group.
Cores 0-3 handle batch 0, cores 4-7 batch 1. Attention is tensor-parallel
(rank r owns heads 4r..4r+3); a chunked ReduceScatter then hands each rank
ownership of 512 tokens (4 fragments of 128), and the MLP runs
sequence-parallel with full weights streamed from HBM — no second
collective and no final AllGather (the host reassembles output slices).

The PE clock ramps with continuous busy time (0.65 -> 1.2 -> 2.4 GHz), so
independent matmul work (next chunk's qkv, previous chunk's o-proj, first
MLP half-pass) is interleaved into the attention chain's stall points via
a filler queue to keep the tensor engine saturated.
"""

import sys

sys.path.insert(0, "/opt/trn_rl_repo")

import numpy as np
import ml_dtypes

import concourse.bass as bass
import concourse.bacc as bacc
import concourse.mybir as mybir
import concourse.tile as tile
from concourse.bass_utils import run_bass_kernel_spmd

BF = ml_dtypes.bfloat16
F32 = mybir.dt.float32
BF16 = mybir.dt.bfloat16

D = 1024
NH = 16
DH = 64
MULT = 4
EPS = 1e-5
ROPE_BASE = 10000.0
B = 2
TP = 4  # ranks per group
HPC = NH // TP  # heads per core = 4
QKF = 2 * HPC * DH  # q+k shard features = 512
VF = HPC * DH  # v shard features = 256
MID = MULT * D  # full mlp rows = 4096 (sequence-parallel MLP)
AF = mybir.ActivationFunctionType
ALU = mybir.AluOpType


def build_nc(T):
    """Build the SPMD graph for one core (token count T per batch)."""
    DC = D // 128  # d chunks = 8
    TT = T // 128  # token tiles = 16
    QT = 512  # q-tile width == RS chunk width
    NQ = T // QT  # 4
    CPQ = QT // 128  # 4
    MB = MID // 128  # mlp row blocks = 32
    NT = D // 512
    OW = T // TP  # owned tokens = 512
    OT = OW // 128  # owned token tiles = 4

    nc = bacc.Bacc("TRN2", target_bir_lowering=False, debug=False, num_devices=8)

    x_e = nc.dram_tensor("x", [T, D], F32, kind="ExternalInput")
    qkw_e = nc.dram_tensor("qkw_t", [D, QKF], BF16, kind="ExternalInput")
    vw_e = nc.dram_tensor("vw_m", [D, VF], BF16, kind="ExternalInput")
    ow_e = nc.dram_tensor("ow_m", [VF, D], BF16, kind="ExternalInput")
    w1w_e = nc.dram_tensor("w1t", [128, MB * D], BF16, kind="ExternalInput")
    wgw_e = nc.dram_tensor("wgt", [128, MB * D], BF16, kind="ExternalInput")
    w2w_e = nc.dram_tensor("w2t", [128, DC * MID], BF16, kind="ExternalInput")
    cos_e = nc.dram_tensor("cosr", [128, T], BF16, kind="ExternalInput")
    sin_e = nc.dram_tensor("sinr", [128, T], BF16, kind="ExternalInput")
    cm_e = nc.dram_tensor("cmask", [CPQ * 128, QT], BF16, kind="ExternalInput")
    id_e = nc.dram_tensor("ident", [128, 128], BF16, kind="ExternalInput")
    sel_e = nc.dram_tensor("selc", [256, 128], BF16, kind="ExternalInput")
    out_e = nc.dram_tensor("out", [OW, D], F32, kind="ExternalOutput")

    groups = [[0, 1, 2, 3], [4, 5, 6, 7]]

    with tile.TileContext(nc) as tc:
        with (
            tc.tile_pool(name="const", bufs=1) as cpool,
            tc.tile_pool(name="actfm", bufs=1) as fmpool,
            tc.tile_pool(name="qko", bufs=1) as qkpool,
            tc.tile_pool(name="vaug", bufs=1) as vpool,
            tc.tile_pool(name="frag", bufs=1) as fpool,
            tc.tile_pool(name="xin", bufs=2) as xpool,
            tc.tile_pool(name="xnb", bufs=3) as xnpool,
            tc.tile_pool(name="work", bufs=4) as wpool,
            tc.tile_pool(name="rope", bufs=2) as rpool,
            tc.tile_pool(name="stats", bufs=8) as spool,
            tc.tile_pool(name="psA", bufs=4, space="PSUM") as psA,
            tc.tile_pool(name="psO", bufs=2, space="PSUM") as psO,
            tc.tile_pool(name="psS", bufs=2, space="PSUM") as psS,
            tc.tile_pool(name="dram", bufs=1, space="DRAM") as dpool,
        ):
            # ---- resident weights / tables ----
            def load_tiles(src, width, n, dt=BF16):
                ts = []
                for i in range(n):
                    t = cpool.tile(
                        [128, width], dt, tag=f"{src.name}_{i}", name=f"{src.name}_{i}"
                    )
                    nc.sync.dma_start(t[:], src[i * 128 : (i + 1) * 128, :])
                    ts.append(t)
                return ts

            qkw = load_tiles(qkw_e, QKF, DC)
            vw = load_tiles(vw_e, VF, DC)
            ow = load_tiles(ow_e, D, VF // 128)
            cosr = load_tiles(cos_e, T, 1)[0]
            sinr = load_tiles(sin_e, T, 1)[0]
            nmask = load_tiles(cm_e, QT, CPQ)
            ident = load_tiles(id_e, 128, 1)[0]
            epsc = cpool.tile([128, 1], F32, tag="epsc", name="epsc")
            nc.vector.memset(epsc[:], EPS)
            # head-select tiles for denominator broadcast:
            # sel[ot][p, c] = 1 iff p == 32*(2*ot + c//64) (host-precomputed)
            sel = []
            for ot in range(2):
                s = cpool.tile([128, 128], BF16, tag=f"sel{ot}", name=f"sel{ot}")
                nc.sync.dma_start(s[:], sel_e[ot * 128 : (ot + 1) * 128, :])
                sel.append(s)

            rs_in = dpool.tile([T, D], BF16, name="rs_in")
            rs_out = dpool.tile([OW, D], BF16, name="rs_out")

            # warmup collective: full-size dummy RS so the first real chunk
            # doesn't pay the large-transfer CC setup cost
            wu_in = dpool.tile([QT, D], BF16, name="wu_in")
            wu_out = dpool.tile([QT // TP, D], BF16, name="wu_out")
            nc.gpsimd.collective_compute(
                "ReduceScatter",
                ALU.add,
                ins=[wu_in[:, :].opt()],
                outs=[wu_out[:, :].opt()],
                replica_groups=groups,
            )

            # ---- persistent activation tiles ----
            xnf = [
                fmpool.tile([128, T], BF16, tag=f"fm{d}", name=f"xnf{d}")
                for d in range(DC)
            ]
            q_sb = [
                qkpool.tile([128, T], BF16, tag=f"qk{i}", name=f"q{i}")
                for i in range(2)
            ]
            k_sb = [
                qkpool.tile([128, T], BF16, tag=f"qk{i + 2}", name=f"k{i}")
                for i in range(2)
            ]
            O_sb = [
                qkpool.tile([128, T], BF16, tag=f"qk{i + 4}", name=f"O{i}")
                for i in range(2)
            ]
            On_sb = [
                qkpool.tile([128, T], BF16, tag=f"qk{i + 6}", name=f"On{i}")
                for i in range(2)
            ]
            v_aug = [
                vpool.tile([128, HPC, DH + 1], BF16, tag=f"va{ti}", name=f"va{ti}")
                for ti in range(TT)
            ]
            # owned-token tiles (sequence-parallel MLP)
            hnf = [
                fpool.tile([128, OW], BF16, tag=f"hn{d}", name=f"hnf{d}")
                for d in range(DC)
            ]
            h1f = [
                fpool.tile([128, D], BF16, tag=f"h1f{k}", name=f"h1f{k}")
                for k in range(OT)
            ]
            out_sb = [
                fpool.tile([128, D], BF16, tag=f"os{k}", name=f"out_sb{k}")
                for k in range(OT)
            ]

            # ---- filler queue: independent work pumped into attn stalls ----
            fillers = []

            def pump():
                if fillers:
                    fillers.pop(0)()

            def drain():
                while fillers:
                    fillers.pop(0)()

            # ---- helpers ----
            def norm_into_fm(xt, fm_tiles, ti):
                """rmsnorm the token tile xt, write bf16 feature-major at col ti."""
                ss = spool.tile([128, 1], F32, tag="ss", name="ss")
                sq = xnpool.tile([128, D], BF16, tag="sq", name="sq", bufs=2)
                nc.scalar.activation(
                    out=sq[:], in_=xt[:], func=AF.Square, accum_out=ss[:]
                )
                sr = spool.tile([128, 1], F32, tag="sr", name="sr")
                nc.scalar.activation(
                    out=sr[:], in_=ss[:], func=AF.Sqrt, bias=epsc[:], scale=1.0 / D
                )
                s1 = spool.tile([128, 1], F32, tag="s1", name="s1")
                nc.vector.reciprocal(s1[:], sr[:])
                xn = xnpool.tile([128, D], BF16, tag="xn", name="xn")
                nc.vector.tensor_scalar_mul(xn[:], xt[:], s1[:])
                for di in range(DC):
                    tp = psS.tile([128, 128], BF16, tag="tp", name="tp", bufs=2)
                    nc.tensor.transpose(
                        tp[:], xn[:, di * 128 : (di + 1) * 128], ident[:]
                    )
                    nc.any.tensor_copy(
                        fm_tiles[di][:, ti * 128 : (ti + 1) * 128], tp[:]
                    )

            def norm_tile(ti):
                xt = xpool.tile([128, D], F32, tag="xt", name="xt")
                nc.sync.dma_start(xt[:], x_e[ti * 128 : (ti + 1) * 128, :])
                norm_into_fm(xt, xnf, ti)

            def qk_m(t4, m):  # one of q01 q23 k01 k23 for chunk t4
                tsl = slice(t4 * QT, (t4 + 1) * QT)
                dst = q_sb[m] if m < 2 else k_sb[m - 2]
                ps = psA.tile([128, 512], F32, tag="ps", name="ps")
                for dc in range(DC):
                    nc.tensor.matmul(
                        ps[:, :QT],
                        qkw[dc][:, m * 128 : (m + 1) * 128],
                        xnf[dc][:, tsl],
                        start=(dc == 0),
                        stop=(dc == DC - 1),
                    )
                qb = rpool.tile([128, QT], BF16, tag="qb", name="qb")
                nc.vector.tensor_copy(qb[:], ps[:, :QT])
                rot = rpool.tile([128, QT], BF16, tag="rot", name="rot")
                for hb in (0, 64):
                    nc.vector.tensor_scalar_mul(
                        rot[hb : hb + 32, :], qb[hb + 32 : hb + 64, :], -1.0
                    )
                    nc.vector.tensor_copy(
                        rot[hb + 32 : hb + 64, :], qb[hb : hb + 32, :]
                    )
                t1 = rpool.tile([128, QT], BF16, tag="t1", name="t1")
                nc.vector.tensor_mul(t1[:], qb[:], cosr[:, tsl])
                t2 = rpool.tile([128, QT], BF16, tag="t2", name="t2")
                nc.vector.tensor_mul(t2[:], rot[:], sinr[:, tsl])
                nc.vector.tensor_add(dst[:, tsl], t1[:], t2[:])

            def v_ti(ti):
                ps = psS.tile([128, VF], F32, tag="tp", name="psv")
                for dc in range(DC):
                    nc.tensor.matmul(
                        ps[:],
                        xnf[dc][:, ti * 128 : (ti + 1) * 128],
                        vw[dc][:],
                        start=(dc == 0),
                        stop=(dc == DC - 1),
                    )
                va = v_aug[ti]
                nc.vector.tensor_copy(
                    va[:, :, 0:DH], ps.rearrange("p (h d) -> p h d", h=HPC)
                )
                nc.vector.memset(va[:, :, DH : DH + 1], 1.0)

            # ---- attention ----
            def attn_qtile(qt):
                tsl = slice(qt * QT, (qt + 1) * QT)
                ncks = CPQ * (qt + 1)
                recf = spool.tile([128, QT], F32, tag="recf", name="recf", bufs=2)
                nc.vector.memset(recf[:], 1.0)
                for hp in range(2):
                    opsP = [
                        psO.tile([DH + 1, QT], F32, tag="pso", name=f"ops{i}")
                        for i in range(2)
                    ]
                    for ck in range(ncks):
                        j = ck - CPQ * qt
                        pts = []
                        for i in range(2):
                            hb = i * 64
                            sp = psA.tile([128, 512], F32, tag="ps", name="sp")
                            nc.tensor.matmul(
                                sp[:, :QT],
                                k_sb[hp][hb : hb + DH, ck * 128 : (ck + 1) * 128],
                                q_sb[hp][hb : hb + DH, tsl],
                                start=True,
                                stop=(j < 0),
                            )
                            if j >= 0:  # fold causal mask into the psum
                                nc.tensor.matmul(
                                    sp[:, :QT],
                                    ident[:],
                                    nmask[j][:],
                                    start=False,
                                    stop=True,
                                )
                            pt = wpool.tile(
                                [128, QT], BF16, tag="pt", name="pt", bufs=4
                            )
                            nc.scalar.activation(
                                out=pt[:], in_=sp[:, :QT], func=AF.Exp, scale=0.125
                            )
                            pts.append(pt)
                        pump()
                        for i in range(2):
                            nc.tensor.matmul(
                                opsP[i][:],
                                v_aug[ck][:, 2 * hp + i, :],
                                pts[i][:],
                                start=(ck == 0),
                                stop=(ck == ncks - 1),
                            )
                    for i in range(2):
                        h = 2 * hp + i
                        nc.vector.tensor_copy(
                            recf[32 * h : 32 * h + 1, :], opsP[i][DH : DH + 1, :]
                        )
                        nc.scalar.copy(
                            O_sb[hp][i * 64 : i * 64 + DH, tsl], opsP[i][0:DH, :]
                        )
                rec32 = spool.tile([128, QT], F32, tag="rec32", name="rec32", bufs=2)
                nc.vector.reciprocal(rec32[:], recf[:])
                recb = spool.tile([128, QT], BF16, tag="recb", name="recb", bufs=2)
                nc.gpsimd.tensor_copy(recb[:], rec32[:])
                return recb

            def normalize_qt(qt, recb):
                tsl = slice(qt * QT, (qt + 1) * QT)
                for ot in range(2):
                    bb = psA.tile([128, 512], F32, tag="ps", name="bb")
                    nc.tensor.matmul(
                        bb[:, :QT], sel[ot][:], recb[:], start=True, stop=True
                    )
                    nc.vector.tensor_mul(
                        On_sb[ot][:, tsl], O_sb[ot][:, tsl], bb[:, :QT]
                    )

            def oproj_ti(ti):  # o-proj partial + x/TP for one 128-token tile
                ob = wpool.tile([128, D], BF16, tag="ob", name="ob", bufs=3)
                xo = xpool.tile([128, D], F32, tag="xo", name="xo")
                nc.sync.dma_start(xo[:], x_e[ti * 128 : (ti + 1) * 128, :])
                for nt in range(NT):
                    ps = psA.tile([128, 512], F32, tag="ps", name="ps")
                    for c in range(VF // 128):
                        nc.tensor.matmul(
                            ps[:, :512],
                            On_sb[c][:, ti * 128 : (ti + 1) * 128],
                            ow[c][:, nt * 512 : (nt + 1) * 512],
                            start=(c == 0),
                            stop=(c == VF // 128 - 1),
                        )
                    nc.vector.scalar_tensor_tensor(
                        ob[:, nt * 512 : (nt + 1) * 512],
                        xo[:, nt * 512 : (nt + 1) * 512],
                        1.0 / TP,
                        ps[:, :512],
                        ALU.mult,
                        ALU.add,
                    )
                nc.sync.dma_start(rs_in[ti * 128 : (ti + 1) * 128, :], ob[:])

            def rs_fire(qt):
                nc.gpsimd.collective_compute(
                    "ReduceScatter",
                    ALU.add,
                    ins=[rs_in[qt * QT : (qt + 1) * QT, :].opt()],
                    outs=[rs_out[qt * 128 : (qt + 1) * 128, :].opt()],
                    replica_groups=groups,
                )

            def resid_frag(qt):
                """h1 fragment qt (128 owned tokens) -> norm2 feature-major."""
                h1 = h1f[qt]
                nc.gpsimd.dma_start(h1[:], rs_out[qt * 128 : (qt + 1) * 128, :])
                norm_into_fm(h1, hnf, qt)

            # ---- MLP pieces (sequence-parallel, full weights) ----
            a_t = [
                fmpool.tile([128, T], BF16, tag=f"fm{j}", name=f"a{j}")
                for j in range(DC)
            ]

            def a_slice(mc, cols=slice(0, OW)):
                base = (mc % 4) * OW
                return a_t[mc // 4][:, base + cols.start : base + cols.stop]

            def mlp_block(mc, half):
                """wg/w1/silu/mul for row-block mc, token cols half*256..+256."""
                cols = slice(half * (OW // 2), (half + 1) * (OW // 2))
                CW = OW // 2
                wgb = wpool.tile([128, D], BF16, tag="wgs", name="wgb", bufs=3)
                nc.sync.dma_start(wgb[:], wgw_e[:, mc * D : (mc + 1) * D])
                w1b = wpool.tile([128, D], BF16, tag="w1s", name="w1b", bufs=3)
                nc.sync.dma_start(w1b[:], w1w_e[:, mc * D : (mc + 1) * D])
                psg = psA.tile([128, 512], F32, tag="ps", name="psg")
                for dc in range(DC):
                    nc.tensor.matmul(
                        psg[:, :CW],
                        wgb[:, dc * 128 : (dc + 1) * 128],
                        hnf[dc][:, cols],
                        start=(dc == 0),
                        stop=(dc == DC - 1),
                    )
                g_sb = wpool.tile([128, CW], BF16, tag="g", name="g", bufs=2)
                nc.scalar.activation(out=g_sb[:], in_=psg[:, :CW], func=AF.Silu)
                psu = psA.tile([128, 512], F32, tag="ps", name="psu")
                for dc in range(DC):
                    nc.tensor.matmul(
                        psu[:, :CW],
                        w1b[:, dc * 128 : (dc + 1) * 128],
                        hnf[dc][:, cols],
                        start=(dc == 0),
                        stop=(dc == DC - 1),
                    )
                nc.vector.tensor_mul(a_slice(mc, cols), g_sb[:], psu[:, :CW])

            # ---- schedule ----
            # front: chunk 0 norm + qkv
            for ti in range(CPQ):
                norm_tile(ti)
            for m in range(4):
                qk_m(0, m)
            for ti in range(CPQ):
                v_ti(ti)

            for qt in range(NQ):
                nxt = qt + 1
                if nxt < NQ:
                    for ti in range(nxt * CPQ, (nxt + 1) * CPQ):
                        fillers.append(lambda ti=ti: norm_tile(ti))
                    for m in range(4):
                        fillers.append(lambda m=m, nxt=nxt: qk_m(nxt, m))
                    for ti in range(nxt * CPQ, (nxt + 1) * CPQ):
                        fillers.append(lambda ti=ti: v_ti(ti))
                if qt == 2:
                    fillers.append(lambda: resid_frag(0))
                if qt == 3:
                    fillers.append(lambda: resid_frag(1))
                    # part of the first MLP half-pass (cols 0:256 <- frags 0,1)
                    for mc in range(12):
                        fillers.append(lambda mc=mc: mlp_block(mc, 0))
                recb = attn_qtile(qt)
                normalize_qt(qt, recb)
                # o-proj + RS on the critical path: fire the chunk ASAP
                for ti in range(qt * CPQ, (qt + 1) * CPQ):
                    oproj_ti(ti)
                rs_fire(qt)
                if qt < 3:
                    drain()

            drain()  # leftover first-half MLP blocks from the qt3 round
            resid_frag(2)
            for mc in range(12, MB):  # rest of first half-pass (bridges RS3)
                mlp_block(mc, 0)
            resid_frag(3)
            for mc in range(MB):  # second half-pass (cols 256:512 <- frags 2,3)
                mlp_block(mc, 1)

            # w2: feature-major out, then PE-transpose back to token-major.
            # Transposes for block `do` are emitted after the po-accumulation
            # of block do+1 so the PE never waits on the scalar pob copy.
            pend = []

            def w2_flush():
                while pend:
                    pob, do = pend.pop(0)
                    for tt in range(OT):
                        tp = psS.tile([128, 128], BF16, tag="tp", name="tp", bufs=2)
                        nc.tensor.transpose(
                            tp[:], pob[:, tt * 128 : (tt + 1) * 128], ident[:]
                        )
                        nc.any.tensor_copy(
                            out_sb[tt][:, do * 128 : (do + 1) * 128], tp[:]
                        )

            for do in range(DC):
                po = psA.tile([128, 512], F32, tag="ps", name="po")
                for half in range(2):
                    w2b = wpool.tile(
                        [128, MID // 2], BF16, tag="w2s", name="w2b", bufs=2
                    )
                    nc.sync.dma_start(
                        w2b[:],
                        w2w_e[
                            :,
                            do * MID + half * MID // 2 : do * MID + (half + 1) * MID // 2,
                        ],
                    )
                    for jj in range(MB // 2):
                        mc = half * MB // 2 + jj
                        nc.tensor.matmul(
                            po[:, :OW],
                            w2b[:, jj * 128 : (jj + 1) * 128],
                            a_slice(mc),
                            start=(mc == 0),
                            stop=(mc == MB - 1),
                        )
                pob = wpool.tile([128, OW], BF16, tag="pob", name="pob", bufs=3)
                nc.scalar.copy(pob[:], po[:, :OW])
                pend.append((pob, do))
                if do >= 1:
                    w2_flush()
            w2_flush()

            for tt in range(OT):
                ot = xpool.tile([128, D], F32, tag="xt", name="ot")
                nc.vector.tensor_add(ot[:], out_sb[tt][:], h1f[tt][:])
                nc.sync.dma_start(out_e[tt * 128 : (tt + 1) * 128, :], ot[:])

    nc.compile()
    return nc


def make_in_maps(x, n1_w, n2_w, qkv_w, o_w, w1_w, wg_w, w2_w, T):
    QT = 512
    CPQ = QT // 128
    half = DH // 2
    freqs = np.arange(half, dtype=np.float64) / half
    theta = 1.0 / ROPE_BASE**freqs
    ang = np.arange(T, dtype=np.float64)[:, None] * theta[None, :]  # [T, 32]
    p = np.arange(128) % half
    cosr = np.cos(ang)[:, p].T.astype(BF)  # [128, T]
    sinr = np.sin(ang)[:, p].T.astype(BF)
    # additive causal mask: 0 where allowed, -800 where masked
    cm = np.zeros((CPQ * 128, QT), dtype=BF)
    for j in range(CPQ):
        tk = np.arange(128)[:, None]
        tq = np.arange(QT)[None, :]
        cm[j * 128 : (j + 1) * 128] = np.where(tq >= j * 128 + tk, 0.0, -800.0).astype(
            BF
        )

    # head-select const: selc[ot*128 + p, c] = 1 iff p == 32*(2*ot + c//64)
    selc = np.zeros((256, 128), dtype=BF)
    for ot in range(2):
        for h in range(2):
            selc[ot * 128 + 32 * (2 * ot + h), h * 64 : (h + 1) * 64] = 1.0

    # full MLP weights, tiled for contiguous [128, D] / [128, MID] streams
    MB = MULT * D // 128  # 32
    DCn = D // 128  # 8
    Wg = (wg_w * n2_w[None, :]).T.astype(BF)  # [D, 4096]
    W1 = (w1_w * n2_w[None, :]).T.astype(BF)
    # tile[p, mc, c*128+j] = W[c*128+p, mc*128+j]
    wgt = Wg.reshape(DCn, 128, MB, 128).transpose(1, 2, 0, 3).reshape(128, MB * D)
    w1t = W1.reshape(DCn, 128, MB, 128).transpose(1, 2, 0, 3).reshape(128, MB * D)
    W2 = w2_w.T.astype(BF)  # [4096, D]
    # tile[p, do, mc, j] = W2[mc*128+p, do*128+j]
    w2t = (
        W2.reshape(MB, 128, DCn, 128)
        .transpose(1, 2, 0, 3)
        .reshape(128, DCn * MB * 128)
    )

    in_maps = []
    for c in range(8):
        b, r = c // 4, c % 4
        qs = slice(r * VF, (r + 1) * VF)
        qr = qkv_w[0 * D :][qs] * n1_w[None, :]
        kr = qkv_w[1 * D :][qs] * n1_w[None, :]
        vr = qkv_w[2 * D :][qs] * n1_w[None, :]
        in_maps.append(
            {
                "x": np.ascontiguousarray(x[b, :T], np.float32),
                "qkw_t": np.ascontiguousarray(
                    np.concatenate([qr, kr], 0).T.astype(BF)
                ),
                "vw_m": np.ascontiguousarray(vr.T.astype(BF)),
                "ow_m": np.ascontiguousarray(o_w[:, qs].T.astype(BF)),
                "w1t": np.ascontiguousarray(w1t),
                "wgt": np.ascontiguousarray(wgt),
                "w2t": np.ascontiguousarray(w2t),
                "cosr": cosr,
                "sinr": sinr,
                "cmask": cm,
                "ident": np.eye(128, dtype=BF),
                "selc": selc,
            }
        )
    return in_maps


_CACHE = {}


def _get_nc(T):
    if T not in _CACHE:
        _CACHE[T] = build_nc(T)
    return _CACHE[T]


def run(inputs, T=2048, trace=False):
    nc = _get_nc(T)
    in_maps = make_in_maps(T=T, **inputs)
    res = run_bass_kernel_spmd(nc, in_maps, core_ids=list(range(8)), trace=trace)
    out = np.empty((B, T, D), np.float32)
    QT = 512
    for c in range(8):
        b, r = c // 4, c % 4
        o = res.results[c]["out"]  # [T//TP, D]: fragment qt at rows qt*128
        for qt in range(T // QT):
            out[b, qt * QT + r * 128 : qt * QT + (r + 1) * 128] = o[
                qt * 128 : (qt + 1) * 128
            ]
    return out, res


def kernel(**inputs):
    out, _ = run(inputs, T=2048)
    return out
